# revision 6
# baseline (speedup 1.0000x reference)
"""Trainium2 Bass kernel for nn_DeepGG (GNN message passing), 8 NeuronCores.

Strategy (edge/dst-sharded, self-contained — all shapes hardcoded):
  * Nodes sharded by id: core k owns nodes [k*6250, (k+1)*6250), padded to
    6272 = 49 windows of 128 dst rows.
  * Edges sorted by dst (host), assigned to the owner core of their dst.
    Within a core, edges grouped by 128-dst windows, padded to a uniform
    M tiles of 128 edges per window so the SPMD program is data-independent.
  * Algebraic reductions:
      segsum((hv@W_src)[src]) = segsum(hv[src]) @ W_src   (gather 64-wide)
      segsum(he@W_edge)       = segsum(he) @ W_edge       (he streamed once)
      softmax(l + base) = softmax(l)                      (base cancels)
  * Per window, segment-sum = matmul with an on-device-built 0/1 segment
    matrix: psum[64,128] += G[:,m,:].T @ segmat[:,m,:], G gathered by
    indirect DMA from the replicated node table.
  * All dense per-node math in feature-major (transposed) layout; per-node
    scalars (1/deg, mask) broadcast across partitions via stride-0 DMA.
  * hv table AllGathered between rounds; one small AllReduce carries the
    graph-level scalars (conv column sums, hv[last], sum(exp logits)).
  * Host does: index prep/sharding, unshard, and the O(H^2) scalar epilogue
    (graph_emb / two 3-logit heads) from the AllReduce payload.
"""
import numpy as np

import concourse.bass as bass
import concourse.bacc as bacc
import concourse.tile as tile
from concourse import mybir
from concourse.bass_utils import run_bass_kernel_spmd

f32 = mybir.dt.float32
i32 = mybir.dt.int32
AF = mybir.ActivationFunctionType
OP = mybir.AluOpType
AX = mybir.AxisListType

N, E, H = 50000, 400000, 64
R = 2
NCORES = 8
P = 128
NSH = 6250                 # real nodes per core
W = 49                     # dst windows per core
NSH_PAD = W * P            # 6272
NPAD = NCORES * NSH_PAD    # 50176
ZROW = NPAD                # all-zero row used by padded edge slots
TBL_ROWS = NPAD + P        # 50304 (tail rows all zero)
CH = 448                   # node-phase chunk (free dim; fits one PSUM bank)
NCH = NSH_PAD // CH        # 14
NCH_FULL = NSH // CH       # 13 full chunks of real nodes (13*448 = 5824)
LASTREAL = NSH - NCH_FULL * CH  # real cols in the last chunk (426)

TRACE = False              # test harness can set kernel.TRACE = True
LAST_RESULTS = {}          # exec metadata for the test harness


def _pad_id(n):
    return (n // NSH) * NSH_PAD + (n % NSH)


def _preprocess(src, dst, he):
    src = np.asarray(src).astype(np.int64)
    dst = np.asarray(dst).astype(np.int64)
    he = np.asarray(he, dtype=np.float32)

    owner = dst // NSH
    dl_global = dst % NSH
    win = dl_global // P
    row = dl_global % P

    key = owner * W + win
    counts = np.bincount(key, minlength=NCORES * W)
    M = int(np.ceil(counts.max() / P))

    order = np.argsort(key, kind="stable")
    src_s, row_s, key_s = src[order], row[order], key[order]
    he_s = he[order]

    group_start = np.zeros(NCORES * W, np.int64)
    group_start[1:] = np.cumsum(counts)[:-1]
    pos = np.arange(E) - group_start[key_s]

    src_idx = np.full((NCORES, W, P, M), ZROW, np.int32)
    dstloc = np.zeros((NCORES, W, P, M), np.float32)
    he_sh = np.zeros((NCORES, W, P, M, H), np.float32)

    c = key_s // W
    w = key_s % W
    m = pos // P
    j = pos % P
    src_idx[c, w, j, m] = _pad_id(src_s).astype(np.int32)
    dstloc[c, w, j, m] = row_s.astype(np.float32)
    he_sh[c, w, j, m] = he_s
    # device layout: [P, W, M] for indices, [W, P, M*H] for he
    return (np.ascontiguousarray(src_idx.transpose(0, 2, 1, 3)),
            np.ascontiguousarray(dstloc.transpose(0, 2, 1, 3)),
            he_sh.reshape(NCORES, W, P, M * H), M)


def build_program(M):
    nc = bacc.Bacc(num_devices=NCORES)

    def inp(name, shape, dtype=f32):
        return nc.dram_tensor(name, shape, dtype, kind="ExternalInput")

    tbl0 = inp("tbl0", [TBL_ROWS, H])
    hv0T = inp("hv0T", [H, NSH_PAD])
    src_idx = inp("src_idx", [P, W, M], i32)
    dstloc = inp("dstloc", [P, W, M])
    he_sh = inp("he_sh", [W, P, M * H])
    invdeg = inp("invdeg", [NSH_PAD])
    maskv = inp("maskv", [NSH_PAD])
    nsv = inp("nsv", [NSH_PAD])
    ndv = inp("ndv", [NSH_PAD])
    lastflag = inp("lastflag", [1])
    wsrc = [inp(f"wsrc{t}", [H, 2 * H]) for t in range(R)]
    wedge = [inp(f"wedge{t}", [H, 2 * H]) for t in range(R)]
    wself = [inp(f"wself{t}", [H, 2 * H]) for t in range(R)]
    bmsg = [inp(f"bmsg{t}", [2 * H]) for t in range(R)]
    wih = [inp(f"wih{t}", [2 * H, 3 * H]) for t in range(R)]
    whh = [inp(f"whh{t}", [H, 3 * H]) for t in range(R)]
    br = [inp(f"br{t}", [H]) for t in range(R)]     # b_ih[:H]+b_hh[:H]
    bz = [inp(f"bz{t}", [H]) for t in range(R)]     # b_ih[H:2H]+b_hh[H:2H]
    bihc = [inp(f"bihc{t}", [H]) for t in range(R)]
    bhhc = [inp(f"bhhc{t}", [H]) for t in range(R)]
    wgate = inp("wgate", [H, 7])
    bgate = inp("bgate", [7])
    wc3 = inp("wc3", [H, 1])

    out_hv = nc.dram_tensor("out_hv", [NSH_PAD, H], f32, kind="ExternalOutput")
    out_probs = nc.dram_tensor("out_probs", [1, NSH_PAD], f32, kind="ExternalOutput")
    out_ar = nc.dram_tensor("out_ar", [P, 1], f32, kind="ExternalOutput")

    bounce_hv = nc.dram_tensor("bounce_hv", [NSH_PAD, H], f32)
    tbl1 = nc.dram_tensor("tbl1", [TBL_ROWS, H], f32, addr_space="Shared")
    bounce_y = nc.dram_tensor("bounce_y", [NSH_PAD, 8], f32)
    ytab = nc.dram_tensor("ytab", [TBL_ROWS, 8], f32, addr_space="Shared")
    ar_in = nc.dram_tensor("ar_in", [P, 1], f32)
    ar_out = nc.dram_tensor("ar_out", [P, 1], f32, addr_space="Shared")

    ident_c = nc.inline_tensor(np.eye(P, dtype=np.float32), "ident_c")
    rg = [list(range(NCORES))]

    def bcast(t, parts, n, off=0):
        """stride-0 partition broadcast of a 1-D DRAM vector slice"""
        a = t[:]
        return bass.AP(tensor=a.tensor, offset=a.offset + off,
                       ap=[[0, parts], [1, n]])

    with tile.TileContext(nc) as tc:
        with (
            tc.tile_pool(name="const", bufs=1) as const,
            tc.tile_pool(name="wp", bufs=2) as wp,
            tc.tile_pool(name="cp", bufs=2) as cp,
        ):
            # ---------------- constants / residents ----------------
            iota_i = cp.tile([P, P], i32, tag="t1")
            nc.gpsimd.iota(iota_i[:], pattern=[[1, P]], base=0, channel_multiplier=0)
            iota_sb = const.tile([P, P], f32)
            nc.vector.tensor_copy(iota_sb[:], iota_i[:])
            ident_sb = const.tile([P, P], f32)
            nc.sync.dma_start(out=ident_sb[:], in_=ident_c[:])
            srcidx_sb = const.tile([P, W, M], i32)
            nc.sync.dma_start(out=srcidx_sb[:], in_=src_idx[:])
            dstloc_d = cp.tile([P, W * M], f32, tag="act")
            nc.sync.dma_start(out=dstloc_d[:], in_=dstloc[:].rearrange("p w m -> p (w m)"))
            dstloc_sb = const.tile([P, W, M], f32)
            nc.vector.tensor_copy(dstloc_sb[:].rearrange("p w m -> p (w m)"), dstloc_d[:])

            invdeg_b = const.tile([P, NSH_PAD], f32)
            nc.gpsimd.dma_start(out=invdeg_b[:], in_=bcast(invdeg, P, NSH_PAD))
            ns_nm = const.tile([P, W], f32)
            nc.sync.dma_start(out=ns_nm[:], in_=nsv[:].rearrange("(w p) -> p w", p=P))
            nd_nm = const.tile([P, W], f32)
            nc.sync.dma_start(out=nd_nm[:], in_=ndv[:].rearrange("(w p) -> p w", p=P))
            lf_sb = const.tile([H, 1], f32)
            nc.gpsimd.dma_start(out=lf_sb[:], in_=bcast(lastflag, H, 1))

            wsrc_sb, wedge_sb, wself_sb, bmsg_sb = [], [], [], []
            wih_sb, whh_sb, br_sb, bz_sb, bihc_sb, bhhc_sb = [], [], [], [], [], []
            for t in range(R):
                for nm, lst, src_t, shp in (
                    ("wsrc", wsrc_sb, wsrc[t], [H, 2 * H]),
                    ("wedge", wedge_sb, wedge[t], [H, 2 * H]),
                    ("wself", wself_sb, wself[t], [H, 2 * H]),
                    ("wih", wih_sb, wih[t], [2 * H, 3 * H]),
                    ("whh", whh_sb, whh[t], [H, 3 * H]),
                ):
                    tl = const.tile(shp, f32, tag=f"{nm}{t}")
                    nc.sync.dma_start(out=tl[:], in_=src_t[:])
                    lst.append(tl)
                for nm, lst, src_t, parts in (
                    ("bmsg", bmsg_sb, bmsg[t], 2 * H),
                    ("br", br_sb, br[t], H),
                    ("bz", bz_sb, bz[t], H),
                    ("bihc", bihc_sb, bihc[t], H),
                    ("bhhc", bhhc_sb, bhhc[t], H),
                ):
                    tl = const.tile([parts, 1], f32, tag=f"{nm}{t}")
                    nc.sync.dma_start(out=tl[:], in_=src_t[:, None])
                    lst.append(tl)
            wgate_sb = const.tile([H, 7], f32)
            nc.sync.dma_start(out=wgate_sb[:], in_=wgate[:])
            bgate_sb = const.tile([7, 1], f32)
            nc.sync.dma_start(out=bgate_sb[:], in_=bgate[:, None])
            wc3_sb = const.tile([H, 1], f32)
            nc.sync.dma_start(out=wc3_sb[:], in_=wc3[:])
            ones_sb = const.tile([P, 1], f32)
            nc.vector.memset(ones_sb[:], 1.0)

            ST = const.tile([H, NSH_PAD], f32)
            HeAggT = const.tile([H, NSH_PAD], f32)
            hvA = const.tile([H, NSH_PAD], f32)      # current hv (transposed)
            nc.sync.dma_start(out=hvA[:], in_=hv0T[:])
            hvB = const.tile([H, NSH_PAD], f32)
            ynm = const.tile([P, W, 8], f32)         # node-major y (shard)
            nc.vector.memset(ynm[:], 0.0)
            caccum = const.tile([P, 8], f32)         # conv accum, node-major
            nc.vector.memset(caccum[:], 0.0)
            partials = const.tile([1, NCH], f32)     # per-chunk sum(exp(l))

            # zero tails of shared gather tables
            ztile = const.tile([P, H], f32)
            nc.vector.memset(ztile[:], 0.0)
            nc.sync.dma_start(out=tbl1[NPAD:TBL_ROWS, :], in_=ztile[:])
            nc.sync.dma_start(out=ytab[NPAD:TBL_ROWS, :], in_=ztile[:, 0:8])

            # ---------------- edge pass helper ----------------
            def edge_pass(table, width, outT, with_he, ps_pool):
                """outT[:, w*P:(w+1)*P] = sum_m G_m.T @ segmat_m  (+ HeAggT)
                outT=None -> conv mode: transpose + nd-scale + accumulate."""
                for w in range(W):
                    segmat = wp.tile([P, M, P], f32, tag="segmat")
                    dl = dstloc_sb[:, w, :]
                    in0 = bass.AP(tensor=dl.tensor, offset=dl.offset,
                                  ap=[dl.ap[0], dl.ap[1], [0, P]])
                    io = iota_sb[:]
                    in1 = bass.AP(tensor=io.tensor, offset=io.offset,
                                  ap=[io.ap[0], [0, M], io.ap[1]])
                    nc.vector.tensor_tensor(out=segmat[:], in0=in0, in1=in1,
                                            op=OP.is_equal)
                    G = wp.tile([P, M, H], f32, tag="G")
                    for m in range(M):
                        nc.gpsimd.indirect_dma_start(
                            out=G[:, m, 0:width], out_offset=None, in_=table[:],
                            in_offset=bass.IndirectOffsetOnAxis(
                                ap=srcidx_sb[:, w, m:m + 1], axis=0))
                    psS = ps_pool.tile([width, P], f32, tag="psS", space="PSUM")
                    for m in range(M):
                        nc.tensor.matmul(psS[:], lhsT=G[:, m, 0:width],
                                         rhs=segmat[:, m, :],
                                         start=(m == 0), stop=(m == M - 1))
                    if outT is not None:
                        nc.vector.tensor_copy(outT[:, w * P:(w + 1) * P], psS[:])
                    else:
                        # conv mode: node-major accumulate with nd scaling
                        cs = cp.tile([8, P], f32, tag="s1")
                        nc.vector.tensor_copy(cs[:], psS[:])
                        psT2 = ps_pool.tile([P, 8], f32, tag="psT2", space="PSUM")
                        nc.tensor.transpose(psT2[:], cs[:], ident_sb[0:8, 0:8])
                        sc = cp.tile([P, 8], f32, tag="ghc")
                        nc.vector.tensor_scalar_mul(sc[:], psT2[:], nd_nm[:, w:w + 1])
                        nc.vector.tensor_tensor(out=caccum[:], in0=caccum[:],
                                                in1=sc[:], op=OP.add)
                    if with_he:
                        Hw = wp.tile([P, M * H], f32, tag="Hw")
                        nc.sync.dma_start(out=Hw[:], in_=he_sh[w])
                        psH = ps_pool.tile([H, P], f32, tag="psH", space="PSUM")
                        for m in range(M):
                            nc.tensor.matmul(psH[:], lhsT=Hw[:, m * H:(m + 1) * H],
                                             rhs=segmat[:, m, :],
                                             start=(m == 0), stop=(m == M - 1))
                        nc.vector.tensor_copy(HeAggT[:, w * P:(w + 1) * P], psH[:])

            # ---------------- GRU node phase ----------------
            def node_phase(t, hv_in, hv_out, ps, dst_dram):
                for k in range(NCH):
                    sl = slice(k * CH, (k + 1) * CH)
                    ps_msg = ps.tile([2 * H, CH], f32, tag="ps_msg", space="PSUM")
                    nc.tensor.matmul(ps_msg[:], lhsT=wsrc_sb[t][:], rhs=ST[:, sl],
                                     start=True, stop=False)
                    nc.tensor.matmul(ps_msg[:], lhsT=wedge_sb[t][:],
                                     rhs=HeAggT[:, sl], start=False, stop=True)
                    ps_self = ps.tile([2 * H, CH], f32, tag="ps_self", space="PSUM")
                    nc.tensor.matmul(ps_self[:], lhsT=wself_sb[t][:],
                                     rhs=hv_in[:, sl], start=True, stop=True)
                    t1 = cp.tile([2 * H, CH], f32, tag="t1")
                    nc.vector.tensor_tensor(out=t1[:], in0=ps_msg[:],
                                            in1=invdeg_b[:, sl], op=OP.mult)
                    nc.vector.tensor_tensor(out=t1[:], in0=t1[:], in1=ps_self[:],
                                            op=OP.add)
                    act = cp.tile([2 * H, CH], f32, tag="act")
                    nc.scalar.activation(act[:], t1[:], AF.Identity,
                                         bias=bmsg_sb[t][:])
                    mk = cp.tile([2 * H, CH], f32, tag="mk")
                    nc.gpsimd.dma_start(out=mk[:],
                                        in_=bcast(maskv, 2 * H, CH, off=k * CH))
                    nc.vector.tensor_tensor(out=act[:], in0=act[:], in1=mk[:],
                                            op=OP.mult)
                    # r and z gates, both on partitions 0:H (base-partition
                    # constraint: all elementwise operands must share base 0)
                    ps_gr = ps.tile([H, CH], f32, tag="ps_gr", space="PSUM")
                    nc.tensor.matmul(ps_gr[:], lhsT=wih_sb[t][:, 0:H],
                                     rhs=act[:], start=True, stop=False)
                    nc.tensor.matmul(ps_gr[:], lhsT=whh_sb[t][:, 0:H],
                                     rhs=hv_in[:, sl], start=False, stop=True)
                    ps_gz = ps.tile([H, CH], f32, tag="ps_gz", space="PSUM")
                    nc.tensor.matmul(ps_gz[:], lhsT=wih_sb[t][:, H:2 * H],
                                     rhs=act[:], start=True, stop=False)
                    nc.tensor.matmul(ps_gz[:], lhsT=whh_sb[t][:, H:2 * H],
                                     rhs=hv_in[:, sl], start=False, stop=True)
                    ps_gic = ps.tile([H, CH], f32, tag="ps_gic", space="PSUM")
                    nc.tensor.matmul(ps_gic[:], lhsT=wih_sb[t][:, 2 * H:3 * H],
                                     rhs=act[:], start=True, stop=True)
                    ps_ghc = ps.tile([H, CH], f32, tag="ps_ghc", space="PSUM")
                    nc.tensor.matmul(ps_ghc[:], lhsT=whh_sb[t][:, 2 * H:3 * H],
                                     rhs=hv_in[:, sl], start=True, stop=True)
                    rt = cp.tile([H, CH], f32, tag="rt")
                    nc.scalar.activation(rt[:], ps_gr[:], AF.Sigmoid,
                                         bias=br_sb[t][:])
                    zt = cp.tile([H, CH], f32, tag="zt")
                    nc.scalar.activation(zt[:], ps_gz[:], AF.Sigmoid,
                                         bias=bz_sb[t][:])
                    ghc = cp.tile([H, CH], f32, tag="ghc")
                    nc.scalar.activation(ghc[:], ps_ghc[:], AF.Identity,
                                         bias=bhhc_sb[t][:])
                    nc.vector.tensor_tensor(out=ghc[:], in0=rt[:], in1=ghc[:],
                                            op=OP.mult)
                    nc.vector.tensor_tensor(out=ghc[:], in0=ps_gic[:], in1=ghc[:],
                                            op=OP.add)
                    cg = cp.tile([H, CH], f32, tag="cg")
                    nc.scalar.activation(cg[:], ghc[:], AF.Tanh, bias=bihc_sb[t][:])
                    d1 = cp.tile([H, CH], f32, tag="d1")
                    nc.vector.tensor_tensor(out=d1[:], in0=hv_in[:, sl], in1=cg[:],
                                            op=OP.subtract)
                    nc.vector.tensor_tensor(out=d1[:], in0=zt[:],
                                            in1=d1[:], op=OP.mult)
                    nc.vector.tensor_tensor(out=hv_out[:, sl], in0=cg[:],
                                            in1=d1[:], op=OP.add)
                # transpose shard to node-major, stream to DRAM per window
                dv = dst_dram[:].rearrange("(w p) h -> p w h", p=P)
                for w in range(W):
                    psTr = ps.tile([P, H], f32, tag="psTr", space="PSUM")
                    nc.tensor.transpose(psTr[:], hv_out[:, w * P:(w + 1) * P],
                                        ident_sb[0:H, 0:H])
                    st = cp.tile([P, H], f32, tag="t1")
                    nc.vector.tensor_copy(st[:], psTr[:])
                    nc.sync.dma_start(out=dv[:, w, :], in_=st[:])

            # ---------------- round 0 ----------------
            with tc.tile_pool(name="pse0", bufs=2, space="PSUM") as pse:
                edge_pass(tbl0, H, ST, True, pse)
            tc.no_sync_barrier()
            with tc.tile_pool(name="psn0", bufs=1, space="PSUM") as psn:
                node_phase(0, hvA, hvB, psn, bounce_hv)
            tc.no_sync_barrier()
            nc.gpsimd.collective_compute(
                "AllGather", OP.bypass, replica_groups=rg,
                ins=[bounce_hv[:]], outs=[tbl1[0:NPAD, :]])

            # ---------------- round 1 ----------------
            with tc.tile_pool(name="pse1", bufs=2, space="PSUM") as pse:
                edge_pass(tbl1, H, ST, False, pse)
            tc.no_sync_barrier()
            with tc.tile_pool(name="psn1", bufs=1, space="PSUM") as psn:
                node_phase(1, hvB, hvA, psn, out_hv)
            tc.no_sync_barrier()

            tc.no_sync_barrier()
            # ---------------- gate / Y table (per window) ----------------
            with tc.tile_pool(name="psy", bufs=2, space="PSUM") as psy:
                for w in range(W):
                    sl = slice(w * P, (w + 1) * P)
                    ps_x = psy.tile([7, P], f32, tag="ps_x", space="PSUM")
                    nc.tensor.matmul(ps_x[:], lhsT=wgate_sb[:], rhs=hvA[:, sl],
                                     start=True, stop=True)
                    xw = cp.tile([7, P], f32, tag="s1")
                    nc.scalar.activation(xw[:], ps_x[:], AF.Sigmoid,
                                         bias=bgate_sb[:])
                    psTy = psy.tile([P, 7], f32, tag="psTy", space="PSUM")
                    nc.tensor.transpose(psTy[:], xw[:], ident_sb[0:7, 0:7])
                    nc.vector.tensor_scalar_mul(ynm[:, w, 0:7], psTy[:],
                                                ns_nm[:, w:w + 1])
                yv = bounce_y[:].rearrange("(w p) h -> p w h", p=P)
                nc.sync.dma_start(out=yv, in_=ynm[:])
            nc.gpsimd.collective_compute(
                "AllGather", OP.bypass, replica_groups=rg,
                ins=[bounce_y[:]], outs=[ytab[0:NPAD, :]])

            tc.no_sync_barrier()
            # ---------------- conv pass + epilogue ----------------
            with tc.tile_pool(name="psc", bufs=2, space="PSUM") as psc:
                edge_pass(ytab, 8, None, False, psc)
                ps_cv = psc.tile([1, 8], f32, tag="ps_l", space="PSUM")
                nc.tensor.matmul(ps_cv[:], lhsT=ones_sb[:], rhs=caccum[:],
                                 start=True, stop=True)
                convrow = const.tile([1, 8], f32)
                nc.vector.tensor_copy(convrow[:], ps_cv[:])

                # choose logits: exp(l) per chunk -> out_probs (unscaled)
                for k in range(NCH):
                    sl = slice(k * CH, (k + 1) * CH)
                    ps_l = psc.tile([1, CH], f32, tag="ps_l", space="PSUM")
                    nc.tensor.matmul(ps_l[:], lhsT=wc3_sb[:], rhs=hvA[:, sl],
                                     start=True, stop=True)
                    ex = cp.tile([1, CH], f32, tag="cg")
                    if k < NCH_FULL:
                        nc.scalar.activation(ex[:], ps_l[:], AF.Exp,
                                             accum_out=partials[:, k:k + 1])
                    else:
                        nc.scalar.activation(ex[:], ps_l[:], AF.Exp)
                        nc.vector.reduce_sum(out=partials[:, k:k + 1],
                                             in_=ex[:, 0:LASTREAL], axis=AX.X)
                    nc.sync.dma_start(out=out_probs[:, sl], in_=ex[:])
                sumexp = const.tile([1, 1], f32)
                nc.vector.reduce_sum(out=sumexp[:], in_=partials[:], axis=AX.X)
                hl = const.tile([H, 1], f32)
                nc.vector.tensor_tensor(out=hl[:], in0=hvA[:, NSH - 1:NSH],
                                        in1=lf_sb[:], op=OP.mult)

                nc.sync.dma_start(out=ar_in[73:P, :], in_=ztile[0:P - 73, 0:1])
                nc.sync.dma_start(out=ar_in[0:8, :], in_=convrow[:])
                nc.sync.dma_start(out=ar_in[8:72, :], in_=hl[:])
                nc.sync.dma_start(out=ar_in[72:73, :], in_=sumexp[:])
                nc.gpsimd.collective_compute(
                    "AllReduce", OP.add, replica_groups=rg,
                    ins=[ar_in[:]], outs=[ar_out[:]])

                arsb = const.tile([P, 1], f32)
                nc.sync.dma_start(out=arsb[:], in_=ar_out[:])
                nc.sync.dma_start(out=out_ar[:], in_=arsb[:])
                sesb = const.tile([1, 1], f32)
                nc.sync.dma_start(out=sesb[:], in_=ar_out[72:73, :])
                rec = const.tile([1, 1], f32)
                nc.vector.reciprocal(rec[:], sesb[:])
                # rescale the unscaled exp chunks in DRAM
                for k in range(NCH):
                    sl = slice(k * CH, (k + 1) * CH)
                    pb = cp.tile([1, CH], f32, tag="d1")
                    nc.sync.dma_start(out=pb[:], in_=out_probs[:, sl])
                    nc.vector.tensor_scalar_mul(pb[:], pb[:], rec[:])
                    nc.sync.dma_start(out=out_probs[:, sl], in_=pb[:])

    nc.finalize()
    return nc


_PROGRAM_CACHE = {}


def kernel(hv, he, src, dst, W_self, W_src, W_edge, b_msg, w_ih, w_hh, b_ih, b_hh,
           w_gate, b_gate, w_conv, b_conv, w_choose, b_choose, w_sn, b_sn,
           w_se, b_se):
    hv = np.asarray(hv, np.float32)
    he = np.asarray(he, np.float32)
    f = lambda x: np.ascontiguousarray(np.asarray(x, np.float32))
    W_self, W_src, W_edge, b_msg = f(W_self), f(W_src), f(W_edge), f(b_msg)
    w_ih, w_hh, b_ih, b_hh = f(w_ih), f(w_hh), f(b_ih), f(b_hh)
    w_gate, b_gate, w_conv, b_conv = f(w_gate), f(b_gate), f(w_conv), f(b_conv)
    w_choose, b_choose = f(w_choose), f(b_choose)
    w_sn, b_sn, w_se, b_se = f(w_sn), f(b_sn), f(w_se), f(b_se)

    src_idx, dstloc, he_sh, M = _preprocess(src, dst, he)

    srcl = np.asarray(src).astype(np.int64)
    dstl = np.asarray(dst).astype(np.int64)
    in_deg = np.bincount(dstl, minlength=N).astype(np.float32)
    out_deg = np.bincount(srcl, minlength=N).astype(np.float32)
    inv_deg = 1.0 / np.maximum(in_deg, 1.0)
    maskv = (in_deg > 0).astype(np.float32)
    nsv = np.where(out_deg > 0, 1.0 / np.sqrt(np.maximum(out_deg, 1.0)),
                   0.0).astype(np.float32)
    ndv = np.where(in_deg > 0, 1.0 / np.sqrt(np.maximum(in_deg, 1.0)),
                   0.0).astype(np.float32)

    def shard_vec(v, fill=0.0):
        o = np.full((NCORES, NSH_PAD), fill, np.float32)
        o[:, :NSH] = v.reshape(NCORES, NSH)
        return o

    invdeg_s = shard_vec(inv_deg, 1.0)
    mask_s = shard_vec(maskv)
    ns_s = shard_vec(nsv)
    nd_s = shard_vec(ndv)

    tbl0 = np.zeros((TBL_ROWS, H), np.float32)
    tbl0[:NPAD].reshape(NCORES, NSH_PAD, H)[:, :NSH] = hv.reshape(NCORES, NSH, H)
    hv0T = np.zeros((NCORES, H, NSH_PAD), np.float32)
    hv0T[:, :, :NSH] = hv.reshape(NCORES, NSH, H).transpose(0, 2, 1)

    if M not in _PROGRAM_CACHE:
        _PROGRAM_CACHE[M] = build_program(M)
    nc = _PROGRAM_CACHE[M]

    common = {"tbl0": tbl0, "wgate": w_gate, "bgate": b_gate[:],
              "wc3": np.ascontiguousarray(w_choose[3 * H:])}
    for t in range(R):
        common[f"wsrc{t}"] = W_src[t]
        common[f"wedge{t}"] = W_edge[t]
        common[f"wself{t}"] = W_self[t]
        common[f"bmsg{t}"] = b_msg[t]
        common[f"wih{t}"] = w_ih[t]
        common[f"whh{t}"] = w_hh[t]
        common[f"br{t}"] = b_ih[t][:H] + b_hh[t][:H]
        common[f"bz{t}"] = b_ih[t][H:2 * H] + b_hh[t][H:2 * H]
        common[f"bihc{t}"] = np.ascontiguousarray(b_ih[t][2 * H:])
        common[f"bhhc{t}"] = np.ascontiguousarray(b_hh[t][2 * H:])

    in_maps = []
    for c in range(NCORES):
        m = dict(common)
        m["hv0T"] = hv0T[c]
        m["src_idx"] = src_idx[c]
        m["dstloc"] = dstloc[c]
        m["he_sh"] = he_sh[c]
        m["invdeg"] = invdeg_s[c]
        m["maskv"] = mask_s[c]
        m["nsv"] = ns_s[c]
        m["ndv"] = nd_s[c]
        m["lastflag"] = np.array([1.0 if c == NCORES - 1 else 0.0], np.float32)
        in_maps.append(m)

    res = run_bass_kernel_spmd(nc, in_maps, core_ids=list(range(NCORES)),
                               trace=TRACE)
    LAST_RESULTS["exec_time_ns"] = res.exec_time_ns
    LAST_RESULTS["mean_exec_time_ns"] = res.mean_exec_time_ns

    hv_out = np.empty((N, H), np.float32)
    probs = np.empty(N, np.float32)
    for c in range(NCORES):
        hv_out[c * NSH:(c + 1) * NSH] = res.results[c]["out_hv"][:NSH]
        probs[c * NSH:(c + 1) * NSH] = res.results[c]["out_probs"][0, :NSH]
    ar = res.results[0]["out_ar"][:, 0]
    conv_sums = ar[0:7]
    hv_last = ar[8:72]

    graph_emb = (conv_sums / np.float32(N)) @ w_conv + b_conv
    graph_emb = graph_emb[None, :].astype(np.float32)
    extra = hv_last[None, :]
    sn = (graph_emb @ w_sn + b_sn).astype(np.float32)
    se = (np.concatenate([graph_emb, extra], 1) @ w_se + b_se).astype(np.float32)
    return (hv_out, graph_emb, probs[None, :].astype(np.float32), sn, se)
